# revision 1
# baseline (speedup 1.0000x reference)
"""Gaussian kernel matrix on 8 Trainium2 NeuronCores.

out = exp(-d2 / (2*sigma^2)),  d2[i,j] = ||x_i||^2 + ||x_j||^2 - 2 x_i.x_j,
sigma^2 = mean(d2) = 2*mean(sq) - 2*||mean(X, axis=0)||^2   (closed form).

Sharding: rows of the 4096x4096 output across 8 cores ([512, 4096] tile per
core) from replicated X^T. Each core computes sq/colsum stats only for its
own 512 rows; an AllGather assembles the full sq row and an AllReduce sums
the column partials + total, so sigma^2 is ready early while the GEMM
streams. GEMM runs in float32r (TF32-like) at 1 cycle/row; epilogue is
ACT Exp(G*scale + bias_i) straight out of PSUM, one DVE multiply with
broadcast exp(-s*sq_j) tiles, then per-tile DMA out.
"""
import numpy as np
import sys

sys.path.insert(0, "/opt/trn_rl_repo")
from concourse import bass, tile, mybir  # noqa: E402
from concourse.bass_utils import run_bass_kernel_spmd  # noqa: E402

N, D, NCORES = 4096, 512, 8
RPC = N // NCORES          # 512 output rows per core
P = 128                    # partitions
KT = D // P                # 4 contraction tiles
NT = RPC // P              # 4 output row-tiles per core
JB = 512                   # j-block width
NB = N // JB               # 8 j-blocks
f32 = mybir.dt.float32
f32r = mybir.dt.float32r
ACTF = mybir.ActivationFunctionType


def _split_waits(nc, max_waits=1):
    """walrus in this image encodes at most one sync-wait per instruction;
    split extras into single-wait NOPs placed just before the instruction."""
    for fn in nc.m.functions:
        for bb in fn.blocks:
            out = []
            for inst in bb.instructions:
                si = inst.sync_info
                if si and si.on_wait and len(si.on_wait) > max_waits:
                    waits = list(si.on_wait)
                    extra, keep = waits[:-max_waits], waits[-max_waits:]
                    for j, w in enumerate(extra):
                        out.append(mybir.InstNoOp(
                            name=f"{inst.name}-ws{j}", engine=inst.engine,
                            sync_info=mybir.SyncInfo(on_wait=[w], on_update=[])))
                    si.on_wait = keep
                out.append(inst)
            bb.instructions = out


def build():
    nc = bass.Bass()
    xt_in = nc.dram_tensor("xt", [D, N], f32r, kind="ExternalInput")
    xtc_in = nc.dram_tensor("xtc", [D, RPC], f32r, kind="ExternalInput")
    out_d = nc.dram_tensor("out", [RPC, N], f32, kind="ExternalOutput")


    with tile.TileContext(nc) as tc:
        with (
            tc.tile_pool(name="xt", bufs=1) as xt_pool,
            tc.tile_pool(name="sqb", bufs=4) as sqb_pool,
            tc.tile_pool(name="out", bufs=6) as out_pool,
            tc.tile_pool(name="ej", bufs=1) as ej_pool,
            tc.tile_pool(name="small", bufs=1) as small_pool,
            tc.tile_pool(name="gpsum", bufs=5, space="PSUM") as gpsum,
            tc.tile_pool(name="ejpsum", bufs=2, space="PSUM") as ejpsum,
            tc.tile_pool(name="mpsum", bufs=1, space="PSUM") as mpsum,
        ):
            # ---- constants ------------------------------------------------
            ones_c = small_pool.tile([P, 1], f32r, tag="ones_c")   # f32r col
            ones_r = small_pool.tile([1, P], f32r, tag="ones_r")   # f32r row
            ones_cf = small_pool.tile([P, 1], f32, tag="ones_cf")  # f32 col
            ones_rf = small_pool.tile([1, P], f32, tag="ones_rf")  # f32 row
            ident = small_pool.tile([1, 1], f32, tag="ident")
            nc.vector.memset(ones_cf[:], 1.0)
            nc.vector.memset(ones_rf[:], 1.0)
            nc.vector.memset(ident[:], 1.0)
            nc.vector.tensor_copy(ones_c[:], ones_cf[:])
            nc.vector.tensor_copy(ones_r[:], ones_rf[:])

            # ---- resident tiles -------------------------------------------
            xts = [xt_pool.tile([P, N], f32r, name=f"xts{k}", tag=f"xts{k}")
                   for k in range(KT)]
            xtc = [xt_pool.tile([P, RPC], f32r, name=f"xtc{k}", tag=f"xtc{k}")
                   for k in range(KT)]
            ejb = [ej_pool.tile([P, JB], f32, name=f"ejb{b}", tag=f"ejb{b}")
                   for b in range(NB)]

            # ---- input DMA: lhsT slab first, then j-major chunks ----------
            for k in range(KT):
                nc.sync.dma_start(xtc[k][:], xtc_in[k * P:(k + 1) * P, :])
            for jb in range(NB):
                sl = slice(jb * JB, (jb + 1) * JB)
                for k in range(KT):
                    nc.sync.dma_start(xts[k][:, sl],
                                      xt_in[k * P:(k + 1) * P, sl])

            # ---- own-rows stats: sq_c, colsum partial, own total ----------
            sqc_row = small_pool.tile([1, RPC], f32, tag="sqc_row")
            sqtot_c = small_pool.tile([1, 1], f32, tag="sqtot_c")
            sqcp = ejpsum.tile([1, RPC], f32, tag="ejp")
            msum = small_pool.tile([P, KT], f32, tag="msum")
            for k in range(KT):
                sqcb = sqb_pool.tile([P, RPC], f32r, name=f"sqcb{k}",
                                     tag="sqb")
                nc.scalar.activation(sqcb[:], xtc[k][:], ACTF.Square)
                nc.tensor.matmul(sqcp[:], ones_c[:], sqcb[:],
                                 start=(k == 0), stop=(k == KT - 1))
                nc.vector.tensor_reduce(msum[:, k:k + 1], xtc[k][:],
                                        axis=mybir.AxisListType.X,
                                        op=mybir.AluOpType.add)
            nc.scalar.activation(sqc_row[:], sqcp[:], ACTF.Copy,
                                 accum_out=sqtot_c[:])

            # ---- full-X stats computed locally ----------------------------
            sq_row = small_pool.tile([1, N], f32r, tag="sq_row")
            sqacc = small_pool.tile([1, NB], f32, tag="sqacc")
            msl = small_pool.tile([P, KT * NB], f32, tag="msl")
            for jb in range(NB):
                sl = slice(jb * JB, (jb + 1) * JB)
                sqp = ejpsum.tile([1, JB], f32, name=f"sqp{jb}", tag="ejp")
                for k in range(KT):
                    sqb = sqb_pool.tile([P, JB], f32r, name=f"sqb_{k}_{jb}",
                                        tag="sqb")
                    nc.gpsimd.tensor_mul(sqb[:], xts[k][:, sl],
                                         xts[k][:, sl])
                    nc.tensor.matmul(sqp[:], ones_c[:], sqb[:],
                                     start=(k == 0), stop=(k == KT - 1))
                    nc.vector.tensor_reduce(
                        msl[:, k * NB + jb:k * NB + jb + 1], xts[k][:, sl],
                        axis=mybir.AxisListType.X, op=mybir.AluOpType.add)
                nc.scalar.activation(sq_row[:, sl], sqp[:], ACTF.Copy,
                                     accum_out=sqacc[:, jb:jb + 1])

            # ---- sigma^2 -> r = 1/sigma^2 = 2s, ns = -s -------------------
            msumf = small_pool.tile([P, KT], f32, tag="msumf")
            for k in range(KT):
                nc.vector.tensor_reduce(msumf[:, k:k + 1],
                                        msl[:, k * NB:(k + 1) * NB],
                                        axis=mybir.AxisListType.X,
                                        op=mybir.AluOpType.add)
            msq = small_pool.tile([P, KT], f32, tag="msq")
            nc.vector.tensor_mul(msq[:], msumf[:], msumf[:])
            msqt = small_pool.tile([P, 1], f32, tag="msqt")
            nc.vector.tensor_reduce(msqt[:], msq[:],
                                    axis=mybir.AxisListType.X,
                                    op=mybir.AluOpType.add)
            sx2p = mpsum.tile([1, 1], f32, tag="m1")
            nc.tensor.matmul(sx2p[:], msqt[:], ones_cf[:],
                             start=True, stop=True)
            sx2 = small_pool.tile([1, 1], f32, tag="sx2")
            nc.vector.tensor_copy(sx2[:], sx2p[:])
            sqt = small_pool.tile([1, 1], f32, tag="sqt")
            nc.vector.tensor_reduce(sqt[:], sqacc[:],
                                    axis=mybir.AxisListType.X,
                                    op=mybir.AluOpType.add)
            t1 = small_pool.tile([1, 1], f32, tag="t1")
            nc.vector.tensor_scalar_mul(t1[:], sqt[:], 2.0 / N)
            t2 = small_pool.tile([1, 1], f32, tag="t2")
            nc.vector.tensor_scalar_mul(t2[:], sx2[:], 2.0 / (float(N) * N))
            sig = small_pool.tile([1, 1], f32, tag="sig")
            nc.vector.tensor_sub(sig[:], t1[:], t2[:])
            r = small_pool.tile([1, 1], f32, tag="r")
            nc.vector.reciprocal(r[:], sig[:])
            ns = small_pool.tile([1, 1], f32, tag="ns")
            nc.vector.tensor_scalar_mul(ns[:], r[:], -0.5)

            # broadcast r and ns to [P, 1] columns
            scale_col = small_pool.tile([P, 1], f32, tag="scale_col")
            ns_col = small_pool.tile([P, 1], f32, tag="ns_col")
            for val, col in ((r, scale_col), (ns, ns_col)):
                pb = mpsum.tile([P, 1], f32, name=f"pb_{col.tensor.name}",
                                tag="m1")
                nc.tensor.matmul(pb[:], ones_rf[:], val[:],
                                 start=True, stop=True)
                nc.vector.tensor_copy(col[:], pb[:])

            # bias_col[t] = -s * sq_i  (transpose own sq slice, scale by ns)
            bias_col = small_pool.tile([P, NT], f32, tag="bias_col")
            for t in range(NT):
                tp = mpsum.tile([P, 1], f32, name=f"tp{t}", tag="m1")
                nc.tensor.transpose(tp[:], sqc_row[:, t * P:(t + 1) * P],
                                    ident[:])
                nc.scalar.activation(bias_col[:, t:t + 1], tp[:], ACTF.Copy,
                                     scale=ns_col[:, 0:1])

            # ej tiles: exp(-s*sq_j) broadcast to [P, JB]
            for jb in range(NB):
                sl = slice(jb * JB, (jb + 1) * JB)
                ep = ejpsum.tile([P, JB], f32, name=f"ep{jb}", tag="ejp")
                nc.tensor.matmul(ep[:], ones_r[:], sq_row[:, sl],
                                 start=True, stop=True)
                nc.scalar.activation(ejb[jb][:], ep[:], ACTF.Exp,
                                     scale=ns_col[:, 0:1])

            # ---- main GEMM + fused epilogue, j-major ----------------------
            for jb in range(NB):
                sl = slice(jb * JB, (jb + 1) * JB)
                for t in range(NT):
                    gp = gpsum.tile([P, JB], f32, name=f"gp_{t}_{jb}",
                                    tag="gp")
                    for k in range(KT):
                        nc.tensor.matmul(
                            gp[:],
                            xtc[k][:, t * P:(t + 1) * P],
                            xts[k][:, sl],
                            start=(k == 0), stop=(k == KT - 1))
                    ot = out_pool.tile([P, JB], f32, name=f"ot_{t}_{jb}",
                                       tag="ot")
                    nc.scalar.activation(ot[:], gp[:], ACTF.Exp,
                                         bias=bias_col[:, t:t + 1],
                                         scale=scale_col[:, 0:1])
                    nc.vector.tensor_mul(ot[:], ot[:], ejb[jb][:])
                    nc.sync.dma_start(out_d[t * P:(t + 1) * P, sl], ot[:])

    _split_waits(nc)
    return nc


_NC = None


def kernel(X: np.ndarray) -> np.ndarray:
    global _NC
    if _NC is None:
        _NC = build()
    XT = np.ascontiguousarray(X.T).astype(np.float32, copy=False)
    in_maps = []
    for c in range(NCORES):
        in_maps.append({
            "xt": XT,
            "xtc": np.ascontiguousarray(XT[:, c * RPC:(c + 1) * RPC]),
        })
    res = run_bass_kernel_spmd(_NC, in_maps, list(range(NCORES))).results
    return np.concatenate([res[c]["out"] for c in range(NCORES)], axis=0)

